# revision 1
# baseline (speedup 1.0000x reference)
"""Trainium2 Bass kernel for nn_CirLinear (soft-NAS mixture of block-circulant
projections of a linear layer's weight, then y = x @ W_mix^T + bias).

Sharding: tensor-parallel over out_features — each of the 8 cores owns a
512-row slice of weight/bias (multiples of 16 keep circulant blocks local),
receives the full x, and produces y[:, :, 512*i : 512*(i+1)].

Per-core algorithm:
  1. softmax(alphas) on device (5 values).
  2. The mixture of circulant projections restricted to one 16x16 block is a
     single linear map on the vectorized block: vec' = M @ vec with M a
     symmetric 256x256 matrix, M = sum_i softmax_i * mask_i / bs_i, where the
     mask_i are constant 0/1 matrices (passed from host; value-independent).
  3. W-shard is brought into a "block-packed" SBUF layout, PE-transposed into
     block-vectorized form, multiplied by M on the TensorEngine (float32r),
     transposed back, and round-tripped through DRAM to produce W_mix^T tiles
     ([in_features on partitions, out_features free]).
  4. GEMM: for each 128-token tile, x is PE-transposed to x^T chunks and
     y_tile = x_tile @ W_mix^T + bias accumulated in PSUM (bias injected as a
     K=1 first matmul). float32r matmuls run at full PE rate for N>=256.
"""

import os
import sys

import numpy as np

if "/opt/trn_rl_repo" not in sys.path:
    sys.path.insert(0, "/opt/trn_rl_repo")

import concourse.bass as bass
import concourse.mybir as mybir
from concourse._compat import not_none as nn
from concourse.tile import TileContext
from concourse.vector_clock import ScopedClock
from concourse.bass_utils import run_bass_kernel_spmd

F32 = mybir.dt.float32
F32R = mybir.dt.float32r

IN_F = 1024
OUT_F = 4096
TOK = 16 * 512  # 8192 tokens
NCORES = 8
OSH = OUT_F // NCORES  # 512 out-features per core
NQ = OSH // 16  # 32 out-blocks per core
NTILES = TOK // 128  # 64 token tiles
KCH = IN_F // 128  # 8 contraction chunks
SEARCH_SPACE = [1, 2, 4, 8, 16]

_MAX_WAITS = 1


class _TC(TileContext):
    """Unmodified TileContext; kept as a hook point."""


def _split_excess_waits(nc: bass.Bass, max_waits: int = 1) -> None:
    """Move excess per-instruction sem-waits onto same-engine nops.

    The installed walrus rejects instructions carrying more than one
    sync-wait ("Too many sync wait commands"), but Tile freely attaches
    several.  Splitting them across nops placed immediately before the
    instruction on the same engine stream is semantically identical.
    """
    for fn in nc.m.functions:
        for bb in fn.blocks:
            out = []
            for inst in bb.instructions:
                si = inst.sync_info
                if si is not None and si.on_wait and len(si.on_wait) > max_waits:
                    waits = list(si.on_wait)
                    extra, keep = waits[:-max_waits], waits[-max_waits:]
                    for i in range(0, len(extra), max_waits):
                        nop = mybir.InstNoOp(
                            name=nc.get_next_instruction_name(), ins=[], outs=[]
                        )
                        nop.engine = inst.engine
                        nop.bass_nofuse = True
                        nop.sync_info = mybir.SyncInfo(
                            on_wait=extra[i : i + max_waits], on_update=[]
                        )
                        nc.register_instruction(nop, overwrite=True)
                        out.append(nop)
                    si.on_wait = keep
                out.append(inst)
            bb.instructions[:] = out


def make_masks() -> np.ndarray:
    """Constant [5, 256, 256] mixing masks (already scaled by 1/bs).

    M[(k,j),(k',j')] for block size bs is 1/bs iff k,k' share a bs-sub-block,
    j,j' share a bs-sub-block, and (k-j)+(k'-j') == 0 (mod bs).  The bs=1 term
    is the identity (original-weight passthrough).
    """
    r = np.arange(16)
    kk, jj, kk2, jj2 = np.meshgrid(r, r, r, r, indexing="ij")
    out = np.zeros((5, 256, 256), dtype=np.float32)
    for i, bs in enumerate(SEARCH_SPACE):
        cond = (
            (kk // bs == kk2 // bs)
            & (jj // bs == jj2 // bs)
            & (((kk - jj) + (kk2 - jj2)) % bs == 0)
        )
        out[i] = cond.reshape(256, 256).astype(np.float32) / bs
    return out


def _funny_dram_ap_k(handle, k: int) -> bass.AP:
    """AP over the k-slice of a [512, 1024] DRAM tensor matching the
    block-packed SBUF tile [128 part = q*4 + p_hi, free = p_lo*16 + j] where
    the DRAM element at (16q + k, 256*p_hi + 16*p_lo + j) maps to
    (part, free).  One DMA per k keeps APs within the 3-dim balancer limit."""
    return bass.AP(handle, k * 1024, [[16384, 32], [256, 4], [1, 256]])


def build_nc(with_wmix_debug: bool = False) -> bass.Bass:
    nc = bass.Bass()

    x_d = nc.dram_tensor("x", [TOK, IN_F], F32, kind="ExternalInput")
    w_d = nc.dram_tensor("w", [OSH, IN_F], F32, kind="ExternalInput")
    al_d = nc.dram_tensor("alphas", [1, 5], F32, kind="ExternalInput")
    b_d = nc.dram_tensor("bias", [1, OSH], F32, kind="ExternalInput")
    masks_d = nc.dram_tensor("masks", [5, 256, 256], F32, kind="ExternalInput")
    ident_d = nc.dram_tensor("ident", [128, 128], F32, kind="ExternalInput")
    y_d = nc.dram_tensor("y", [TOK, OSH], F32, kind="ExternalOutput")
    wmix_d = nc.dram_tensor("wmix_scratch", [OSH, IN_F], F32)
    if with_wmix_debug:
        wmdbg_d = nc.dram_tensor("wmix_dbg", [OSH, IN_F], F32, kind="ExternalOutput")

    with _TC(nc) as tc:
        # ---- persistent tiles ----
        with (
            tc.tile_pool(name="persist", bufs=1) as persist,
            tc.tile_pool(name="psc", bufs=1, space="PSUM") as psc,
        ):
            ident = persist.tile([128, 128], F32, tag="ident")
            nc.sync.dma_start(ident[:, :], ident_d[:, :])
            ones_f32 = persist.tile([1, 128], F32, tag="ones_f32")
            nc.vector.memset(ones_f32[:, :], 1.0)
            ones = persist.tile([1, 128], F32R, tag="ones")
            nc.vector.tensor_copy(ones[:, :], ones_f32[:, :])
            bias_f32 = persist.tile([1, OSH], F32, tag="bias_f32")
            nc.sync.dma_start(bias_f32[:, :], b_d[:, :])
            bias_sb = persist.tile([1, OSH], F32R, tag="bias")
            nc.vector.tensor_copy(bias_sb[:, :], bias_f32[:, :])
            wmt = [persist.tile([128, OSH], F32R, tag=f"wmt{kc}", name=f"wmt{kc}")
                   for kc in range(KCH)]

            # ---- phase A: softmax coefficients ----
            with tc.tile_pool(name="coef", bufs=1) as coefp:
                al = coefp.tile([1, 5], F32, tag="al")
                nc.sync.dma_start(al[:, :], al_d[:, :])
                mx = coefp.tile([1, 1], F32, tag="mx")
                nc.vector.reduce_max(mx[:, :], al[:, :], axis=mybir.AxisListType.X)
                nmx = coefp.tile([1, 1], F32, tag="nmx")
                nc.vector.tensor_scalar_mul(nmx[:, :], mx[:, :], -1.0)
                ex = coefp.tile([1, 5], F32, tag="ex")
                nc.scalar.activation(
                    ex[:, :], al[:, :], mybir.ActivationFunctionType.Exp,
                    bias=nmx[:, :], scale=1.0,
                )
                sm = coefp.tile([1, 1], F32, tag="sm")
                nc.vector.reduce_sum(sm[:, :], ex[:, :], axis=mybir.AxisListType.X)
                rs = coefp.tile([1, 1], F32, tag="rs")
                nc.vector.reciprocal(rs[:, :], sm[:, :])
                probs = coefp.tile([1, 5], F32, tag="probs")
                nc.vector.tensor_scalar(
                    probs[:, :], ex[:, :], rs[:, :], None, mybir.AluOpType.mult
                )
                # broadcast coefficients to all 128 partitions via K=1 matmul
                cb_ps = psc.tile([128, 5], F32)
                nc.tensor.matmul(cb_ps[:, :], ones_f32[:, :], probs[:, :])
                cb = persist.tile([128, 5], F32, tag="cb")
                nc.vector.tensor_copy(cb[:, :], cb_ps[:, :])

            # ---- phase B: build M [256,256] as two [128,256] row-halves ----
            mc = [persist.tile([128, 256], F32, tag=f"mc{h}", name=f"mc{h}") for h in range(2)]
            with tc.tile_pool(name="mbuild", bufs=2) as mb:
                for h in range(2):
                    acc = None
                    for i in range(5):
                        msk = mb.tile([128, 256], F32, tag="msk")
                        nc.gpsimd.dma_start(
                            msk[:, :], masks_d[i, h * 128 : (h + 1) * 128, :]
                        )
                        if acc is None:
                            acc = mb.tile([128, 256], F32, tag="acc")
                            nc.vector.tensor_scalar(
                                acc[:, :], msk[:, :], cb[:, 0:1], None,
                                mybir.AluOpType.mult,
                            )
                        else:
                            dst = mc[h] if i == 4 else mb.tile(
                                [128, 256], F32, tag="acc", name="acc"
                            )
                            nc.vector.scalar_tensor_tensor(
                                dst[:, :], msk[:, :], cb[:, i : i + 1], acc[:, :],
                                mybir.AluOpType.mult, mybir.AluOpType.add,
                            )
                            acc = dst

            # ---- phase C: W_mix via blockvec + M matmul ----
            with (
                tc.tile_pool(name="wmixp", bufs=1) as wp,
                tc.tile_pool(name="pst", bufs=3, space="PSUM") as pst,
            ):
                wf = wp.tile([128, 4096], F32, tag="wf")
                wf4 = wf[:, :].rearrange("p (k pl j) -> p k pl j", k=16, pl=16, j=16)
                for k in range(16):
                    nc.gpsimd.dma_start(wf4[:, k, :, :], _funny_dram_ap_k(w_d, k))
                # reorder free dims (k, p_lo, j) -> (p_lo, k, j) so each
                # (k-half, j) transpose input is one contiguous 128-slice
                wr = wp.tile([128, 4096], F32, tag="wr")
                wr4 = wr[:, :].rearrange("p (pl k j) -> p pl k j", pl=16, k=16, j=16)
                nc.vector.tensor_copy(
                    wr4[:, :, :, :],
                    wf4[:, :, :, :].rearrange("p k pl j -> p pl k j"),
                )

                bv = [wp.tile([128, 2048], F32, tag=f"bv{h}", name=f"bv{h}") for h in range(2)]
                for h in range(2):
                    for plg in range(4):  # groups of 4 p_lo per PSUM bank
                        tp = pst.tile([128, 512], F32, tag="pc", name="tp")
                        for pi in range(4):
                            pl = plg * 4 + pi
                            nc.tensor.transpose(
                                tp[:, pi * 128 : (pi + 1) * 128],
                                wr[:, pl * 256 + h * 128 : pl * 256 + (h + 1) * 128],
                                ident[:, :],
                            )
                        nc.vector.tensor_copy(
                            bv[h][:, plg * 512 : (plg + 1) * 512], tp[:, :]
                        )

                bm = [wp.tile([128, 2048], F32, tag=f"bm{h}", name=f"bm{h}") for h in range(2)]
                for hr in range(2):
                    for nch in range(4):
                        mp = pst.tile([128, 512], F32, tag="pc", name="mp")
                        for hc in range(2):
                            nc.tensor.matmul(
                                mp[:, :],
                                mc[hc][:, hr * 128 : (hr + 1) * 128],
                                bv[hc][:, nch * 512 : (nch + 1) * 512],
                                start=(hc == 0),
                                stop=(hc == 1),
                            )
                        nc.scalar.copy(
                            bm[hr][:, nch * 512 : (nch + 1) * 512], mp[:, :]
                        )

                wmf = wp.tile([128, 4096], F32, tag="wmf")
                wmf4 = wmf[:, :].rearrange("p (k pl j) -> p k pl j", k=16, pl=16, j=16)
                for hr in range(2):
                    for plg in range(4):
                        tb = pst.tile([128, 512], F32, tag="pc", name="tb")
                        for pi in range(4):
                            pl = plg * 4 + pi
                            nc.tensor.transpose(
                                tb[:, pi * 128 : (pi + 1) * 128],
                                bm[hr][:, pl * 128 : (pl + 1) * 128],
                                ident[:, :],
                            )
                        src = tb[:, :].rearrange(
                            "p (pl k j) -> p k pl j", pl=4, k=8, j=16
                        )
                        nc.vector.tensor_copy(
                            wmf4[:, hr * 8 : (hr + 1) * 8, plg * 4 : (plg + 1) * 4, :],
                            src,
                        )

                for k in range(16):
                    nc.gpsimd.dma_start(
                        _funny_dram_ap_k(wmix_d, k), wmf4[:, k, :, :]
                    )
                if with_wmix_debug:
                    for k in range(16):
                        nc.gpsimd.dma_start(
                            _funny_dram_ap_k(wmdbg_d, k), wmf4[:, k, :, :]
                        )

                # natural reload + final transposes -> W_mix^T tiles
                nt = [wp.tile([128, IN_F], F32, tag=f"nt{r}", name=f"nt{r}") for r in range(4)]
                for r in range(4):
                    nc.sync.dma_start(
                        nt[r][:, :], wmix_d[r * 128 : (r + 1) * 128, :]
                    )
                for kc in range(KCH):
                    ft = pst.tile([128, 512], F32, tag="pc", name="ft")
                    for r in range(4):
                        nc.tensor.transpose(
                            ft[:, r * 128 : (r + 1) * 128],
                            nt[r][:, kc * 128 : (kc + 1) * 128],
                            ident[:, :],
                        )
                    nc.vector.tensor_copy(wmt[kc][:, :], ft[:, :])

            # ---- phase D: main GEMM over token tiles ----
            with (
                tc.tile_pool(name="xin", bufs=5) as xin,
                tc.tile_pool(name="xtp", bufs=4) as xtp,
                tc.tile_pool(name="yout", bufs=4) as yout,
                tc.tile_pool(name="psx", bufs=2, space="PSUM") as psx,
                tc.tile_pool(name="psy", bufs=3, space="PSUM") as psy,
            ):
                for t in range(NTILES):
                    xt = xin.tile([128, IN_F], F32, tag="xt")
                    nc.scalar.dma_start(xt[:, :], x_d[t * 128 : (t + 1) * 128, :])

                    xps = psx.tile([128, IN_F], F32, tag="xps")
                    for kc in range(KCH):
                        nc.tensor.transpose(
                            xps[:, kc * 128 : (kc + 1) * 128],
                            xt[:, kc * 128 : (kc + 1) * 128],
                            ident[:, :],
                        )
                    xT = xtp.tile([128, IN_F], F32R, tag="xT")
                    nc.vector.tensor_copy(xT[:, 0:512], xps[:, 0:512])
                    nc.scalar.copy(xT[:, 512:1024], xps[:, 512:1024])

                    yps = psy.tile([128, OSH], F32, tag="yps")
                    nc.tensor.matmul(
                        yps[:, :], ones[:, :], bias_sb[:, :], start=True, stop=False
                    )
                    for kc in range(KCH):
                        nc.tensor.matmul(
                            yps[:, :],
                            xT[:, kc * 128 : (kc + 1) * 128],
                            wmt[kc][:, :],
                            start=False,
                            stop=(kc == KCH - 1),
                        )
                    ysb = yout.tile([128, OSH], F32, tag="ysb")
                    if t % 2 == 0:
                        nc.vector.tensor_copy(ysb[:, :], yps[:, :])
                    else:
                        nc.scalar.copy(ysb[:, :], yps[:, :])
                    nc.sync.dma_start(y_d[t * 128 : (t + 1) * 128, :], ysb[:, :])

    _split_excess_waits(nc)
    return nc


_NC_CACHE: dict = {}


def _get_nc(with_wmix_debug: bool = False) -> bass.Bass:
    key = with_wmix_debug
    if key not in _NC_CACHE:
        _NC_CACHE[key] = build_nc(with_wmix_debug)
    return _NC_CACHE[key]


def make_in_maps(x, weight, alphas, bias):
    x2 = np.ascontiguousarray(
        np.asarray(x, dtype=np.float32).reshape(TOK, IN_F)
    )
    weight = np.asarray(weight, dtype=np.float32)
    alphas = np.asarray(alphas, dtype=np.float32).reshape(1, 5)
    bias = np.asarray(bias, dtype=np.float32)
    masks = make_masks()
    ident = np.eye(128, dtype=np.float32)
    in_maps = []
    for c in range(NCORES):
        in_maps.append(
            {
                "x": x2,
                "w": np.ascontiguousarray(weight[c * OSH : (c + 1) * OSH]),
                "alphas": alphas,
                "bias": np.ascontiguousarray(bias[c * OSH : (c + 1) * OSH]).reshape(
                    1, OSH
                ),
                "masks": masks,
                "ident": ident,
            }
        )
    return in_maps


def run(x, weight, alphas, bias, trace=False, with_wmix_debug=False, **rkw):
    nc = _get_nc(with_wmix_debug)
    in_maps = make_in_maps(x, weight, alphas, bias)
    res = run_bass_kernel_spmd(nc, in_maps, list(range(NCORES)), trace=trace, **rkw)
    y = np.concatenate([res.results[c]["y"] for c in range(NCORES)], axis=1)
    y = y.reshape(16, 512, OUT_F)
    return y, res


def kernel(x, weight, alphas, bias):
    y, _ = run(x, weight, alphas, bias)
    return y.astype(np.float32)


if __name__ == "__main__":
    rng = np.random.default_rng(0)
    x = rng.standard_normal((16, 512, IN_F), dtype=np.float32)
    w = (rng.standard_normal((OUT_F, IN_F)) * 0.02).astype(np.float32)
    a = rng.standard_normal(5).astype(np.float32)
    b = (rng.standard_normal(OUT_F) * 0.02).astype(np.float32)
    y = kernel(x=x, weight=w, alphas=a, bias=b)
    print("y", y.shape, y.dtype, float(np.abs(y).max()))



# revision 2
# speedup vs baseline: 1.2600x; 1.2600x over previous
"""Trainium2 Bass kernel for nn_CirLinear (soft-NAS mixture of block-circulant
projections of a linear layer's weight, then y = x @ W_mix^T + bias).

v2 — bf16 + DMA-transpose rewrite.

Sharding: 2-way on tokens x 4-way on out_features (core c: token-half c//4,
out-quarter c%4). Each core gets 4096 tokens and 1024 out-features.

Host precomputes softmax(alphas) and the 256x256 block-mixing matrix M in
float64 (tiny math), ships M as bf16; x and weight are converted to bf16 on
host, with x pre-chunked k-major so the device can load x^T tiles via the
xbar DMA-transpose at full contiguous bandwidth.

Per-core device algorithm (all matmul operands bf16, PSUM accumulation fp32):
  1. x^T tiles [128 k, 4096 tok] via 16 DMA-transposes (no PE involvement).
  2. W_mix construction: funny-DMA W into block-packed layout, DVE riffle,
     PE-transpose to block-vectorized form, one 256-deep M matmul, transpose
     back, funny-DMA to a natural-layout DRAM scratch.
  3. W_mix^T tiles [128 k, 1024 o] via DMA-transpose reads of the scratch.
  4. GEMM: per 128-token tile, 16 matmuls (x^T chunk stationary, W_mix^T
     moving, N=512 per PSUM bank), fused bias-add on DVE, DMA out.
"""

import sys

import numpy as np

if "/opt/trn_rl_repo" not in sys.path:
    sys.path.insert(0, "/opt/trn_rl_repo")

import ml_dtypes

import concourse.bass as bass
import concourse.mybir as mybir
from concourse.tile import TileContext
from concourse.bass_utils import run_bass_kernel_spmd

F32 = mybir.dt.float32
BF16 = mybir.dt.bfloat16
BF16_NP = np.dtype(ml_dtypes.bfloat16)

IN_F = 1024
OUT_F = 4096
TOK = 16 * 512  # 8192 tokens
NCORES = 8
T_SHARD = 2  # token shards
O_SHARD = 4  # out-feature shards
TOKS = TOK // T_SHARD  # 4096 tokens per core
OSH = OUT_F // O_SHARD  # 1024 out-features per core
NQG = 2  # q-groups of 512 weight rows each (OSH = 1024)
NTILES = TOKS // 128  # 32 token tiles
KCH = IN_F // 128  # 8 contraction chunks
SEARCH_SPACE = [1, 2, 4, 8, 16]

_MAX_WAITS = 1


class _TC(TileContext):
    """Unmodified TileContext; kept as a hook point."""


def _split_excess_waits(nc: bass.Bass, max_waits: int = 1) -> None:
    """Move excess per-instruction sem-waits onto same-engine nops.

    The installed walrus rejects instructions carrying more than one
    sync-wait ("Too many sync wait commands"), but Tile freely attaches
    several.  Splitting them across nops placed immediately before the
    instruction on the same engine stream is semantically identical.
    """
    for fn in nc.m.functions:
        for bb in fn.blocks:
            out = []
            for inst in bb.instructions:
                si = inst.sync_info
                if si is not None and si.on_wait and len(si.on_wait) > max_waits:
                    waits = list(si.on_wait)
                    extra, keep = waits[:-max_waits], waits[-max_waits:]
                    for i in range(0, len(extra), max_waits):
                        nop = mybir.InstNoOp(
                            name=nc.get_next_instruction_name(), ins=[], outs=[]
                        )
                        nop.engine = inst.engine
                        nop.bass_nofuse = True
                        nop.sync_info = mybir.SyncInfo(
                            on_wait=extra[i : i + max_waits], on_update=[]
                        )
                        nc.register_instruction(nop, overwrite=True)
                        out.append(nop)
                    si.on_wait = keep
                out.append(inst)
            bb.instructions[:] = out


def make_masks() -> np.ndarray:
    """Constant [5, 256, 256] mixing masks (already scaled by 1/bs).

    M[(k,j),(k',j')] for block size bs is 1/bs iff k,k' share a bs-sub-block,
    j,j' share a bs-sub-block, and (k-j)+(k'-j') == 0 (mod bs).  The bs=1 term
    is the identity (original-weight passthrough).
    """
    r = np.arange(16)
    kk, jj, kk2, jj2 = np.meshgrid(r, r, r, r, indexing="ij")
    out = np.zeros((5, 256, 256), dtype=np.float64)
    for i, bs in enumerate(SEARCH_SPACE):
        cond = (
            (kk // bs == kk2 // bs)
            & (jj // bs == jj2 // bs)
            & (((kk - jj) + (kk2 - jj2)) % bs == 0)
        )
        out[i] = cond.reshape(256, 256).astype(np.float64) / bs
    return out


def _funny_dram_ap(handle, g: int, k: int) -> bass.AP:
    """AP over the (g,k)-slice of a [1024, 1024] DRAM tensor matching the
    block-packed SBUF tile [128 part = q*4 + p_hi, free = p_lo*16 + j] where
    the DRAM element at (512g + 16q + k, 256*p_hi + 16*p_lo + j) maps to
    (part, free).  One DMA per (g,k) keeps APs within the 3-dim limit."""
    return bass.AP(
        handle, g * 512 * 1024 + k * 1024, [[16384, 32], [256, 4], [1, 256]]
    )


def build_nc() -> bass.Bass:
    nc = bass.Bass()

    # x pre-chunked on host: xt[kc, t, i] = x_bf16[t, kc*128 + i]
    xt_d = nc.dram_tensor("xt", [KCH, TOKS, 128], BF16, kind="ExternalInput")
    w_d = nc.dram_tensor("w", [OSH, IN_F], BF16, kind="ExternalInput")
    mc_d = nc.dram_tensor("mc", [2, 128, 256], BF16, kind="ExternalInput")
    b_d = nc.dram_tensor("bias", [1, OSH], F32, kind="ExternalInput")
    ident_d = nc.dram_tensor("ident", [128, 128], BF16, kind="ExternalInput")
    y_d = nc.dram_tensor("y", [TOKS, OSH], F32, kind="ExternalOutput")
    wmix_d = nc.dram_tensor("wmix_scratch", [OSH, IN_F], BF16)

    with _TC(nc) as tc:
        with tc.tile_pool(name="persist", bufs=1) as persist:
            ident = persist.tile([128, 128], BF16, tag="ident")
            nc.sync.dma_start(ident[:, :], ident_d[:, :])
            mc = [
                persist.tile([128, 256], BF16, tag=f"mc{h}", name=f"mc{h}")
                for h in range(2)
            ]
            for h in range(2):
                nc.sync.dma_start(mc[h][:, :], mc_d[h, :, :])
            bias_f32 = persist.tile([1, OSH], F32, tag="bias_f32")
            nc.sync.dma_start(bias_f32[:, :], b_d[:, :])
            ones = persist.tile([1, 128], F32, tag="ones")
            nc.vector.memset(ones[:, :], 1.0)
            bias128 = persist.tile([128, OSH], F32, tag="bias128")
            wmt = [
                persist.tile([128, OSH], BF16, tag=f"wmt{kc}", name=f"wmt{kc}")
                for kc in range(KCH)
            ]
            xT = [
                persist.tile([128, TOKS], BF16, tag=f"xT{kc}", name=f"xT{kc}")
                for kc in range(KCH)
            ]

            # ---- x^T via xbar DMA-transpose (fully contiguous source) ----
            # Split each chunk into token halves so early GEMM tiles unblock
            # while the second half still streams in.
            for half in range(2):
                for kc in range(KCH):
                    nc.sync.dma_start(
                        xT[kc][:, half * 2048 : (half + 1) * 2048],
                        xt_d[kc, half * 2048 : (half + 1) * 2048, :],
                        transpose=True,
                    )

            # ---- W_mix construction ----
            with (
                tc.tile_pool(name="wbuild", bufs=1) as wp,
                tc.tile_pool(name="pst", bufs=3, space="PSUM") as pst,
            ):
                # one-time bias broadcast to 128 partitions (K=1 matmul)
                for h in range(2):
                    pb = pst.tile([128, 512], F32, tag="pc", name="pb")
                    nc.tensor.matmul(
                        pb[:, :],
                        ones[:, :],
                        bias_f32[:, h * 512 : (h + 1) * 512],
                        start=True,
                        stop=True,
                    )
                    nc.vector.tensor_copy(bias128[:, h * 512 : (h + 1) * 512], pb[:, :])

                for g in range(NQG):
                    wf = wp.tile([128, 4096], BF16, tag=f"wf{g}", name=f"wf{g}")
                    wf4 = wf[:, :].rearrange("p (k pl j) -> p k pl j", k=16, pl=16, j=16)
                    for k in range(16):
                        nc.gpsimd.dma_start(wf4[:, k, :, :], _funny_dram_ap(w_d, g, k))
                    # reorder free dims (k, p_lo, j) -> (p_lo, k, j) so each
                    # (k-half, j) transpose input is one contiguous 128-slice
                    wr = wp.tile([128, 4096], BF16, tag=f"wr{g}", name=f"wr{g}")
                    wr4 = wr[:, :].rearrange("p (pl k j) -> p pl k j", pl=16, k=16, j=16)
                    nc.vector.tensor_copy(
                        wr4[:, :, :, :],
                        wf4[:, :, :, :].rearrange("p k pl j -> p pl k j"),
                    )

                    bv = [
                        wp.tile([128, 2048], BF16, tag=f"bv{g}{h}", name=f"bv{g}{h}")
                        for h in range(2)
                    ]
                    for h in range(2):
                        for plg in range(4):
                            tp = pst.tile([128, 512], BF16, tag="pc", name="tp")
                            for pi in range(4):
                                pl = plg * 4 + pi
                                nc.tensor.transpose(
                                    tp[:, pi * 128 : (pi + 1) * 128],
                                    wr[:, pl * 256 + h * 128 : pl * 256 + (h + 1) * 128],
                                    ident[:, :],
                                )
                            if (h * 4 + plg) % 2 == 0:
                                nc.vector.tensor_copy(
                                    bv[h][:, plg * 512 : (plg + 1) * 512], tp[:, :]
                                )
                            else:
                                nc.scalar.copy(
                                    bv[h][:, plg * 512 : (plg + 1) * 512], tp[:, :]
                                )

                    bm = [
                        wp.tile([128, 2048], BF16, tag=f"bm{g}{hr}", name=f"bm{g}{hr}")
                        for hr in range(2)
                    ]
                    for hr in range(2):
                        for nch in range(4):
                            mp = pst.tile([128, 512], F32, tag="pc", name="mp")
                            for hc in range(2):
                                nc.tensor.matmul(
                                    mp[:, :],
                                    mc[hc][:, hr * 128 : (hr + 1) * 128],
                                    bv[hc][:, nch * 512 : (nch + 1) * 512],
                                    start=(hc == 0),
                                    stop=(hc == 1),
                                )
                            nc.scalar.copy(
                                bm[hr][:, nch * 512 : (nch + 1) * 512], mp[:, :]
                            )

                    wmf = wp.tile([128, 4096], BF16, tag=f"wmf{g}", name=f"wmf{g}")
                    wmf4 = wmf[:, :].rearrange(
                        "p (k pl j) -> p k pl j", k=16, pl=16, j=16
                    )
                    for hr in range(2):
                        for plg in range(4):
                            tb = pst.tile([128, 512], BF16, tag="pc", name="tb")
                            for pi in range(4):
                                pl = plg * 4 + pi
                                nc.tensor.transpose(
                                    tb[:, pi * 128 : (pi + 1) * 128],
                                    bm[hr][:, pl * 128 : (pl + 1) * 128],
                                    ident[:, :],
                                )
                            src = tb[:, :].rearrange(
                                "p (pl k j) -> p k pl j", pl=4, k=8, j=16
                            )
                            nc.vector.tensor_copy(
                                wmf4[
                                    :, hr * 8 : (hr + 1) * 8, plg * 4 : (plg + 1) * 4, :
                                ],
                                src,
                            )
                    for k in range(16):
                        nc.gpsimd.dma_start(_funny_dram_ap(wmix_d, g, k), wmf4[:, k, :, :])

                    # W_mix^T tiles for this g-group become available as soon
                    # as its funny-writes land (row range [512g, 512g+512)).
                    for kc in range(KCH):
                        nc.scalar.dma_start(
                            wmt[kc][:, g * 512 : (g + 1) * 512],
                            wmix_d[g * 512 : (g + 1) * 512, kc * 128 : (kc + 1) * 128],
                            transpose=True,
                        )

            # ---- main GEMM over token tiles ----
            with (
                tc.tile_pool(name="yout", bufs=4) as yout,
                tc.tile_pool(name="psy", bufs=3, space="PSUM") as psy,
            ):
                for tt in range(NTILES):
                    yps = psy.tile([128, OSH], F32, tag="yps")
                    for h in range(2):
                        for kc in range(KCH):
                            nc.tensor.matmul(
                                yps[:, h * 512 : (h + 1) * 512],
                                xT[kc][:, tt * 128 : (tt + 1) * 128],
                                wmt[kc][:, h * 512 : (h + 1) * 512],
                                start=(kc == 0),
                                stop=(kc == KCH - 1),
                            )
                    ysb = yout.tile([128, OSH], F32, tag="ysb")
                    nc.vector.scalar_tensor_tensor(
                        ysb[:, :],
                        yps[:, :],
                        1.0,
                        bias128[:, :],
                        mybir.AluOpType.mult,
                        mybir.AluOpType.add,
                    )
                    eng = nc.sync if tt % 2 == 0 else nc.scalar
                    eng.dma_start(y_d[tt * 128 : (tt + 1) * 128, :], ysb[:, :])

    _split_excess_waits(nc)
    return nc


_NC_CACHE: dict = {}


def _get_nc() -> bass.Bass:
    if "nc" not in _NC_CACHE:
        _NC_CACHE["nc"] = build_nc()
    return _NC_CACHE["nc"]


def make_in_maps(x, weight, alphas, bias):
    x2 = np.asarray(x, dtype=np.float32).reshape(TOK, IN_F)
    x_bf = x2.astype(BF16_NP)
    weight_bf = np.asarray(weight, dtype=np.float32).astype(BF16_NP)
    bias = np.asarray(bias, dtype=np.float32)

    # host-side softmax + mixing matrix (float64; rounds once to bf16)
    al = np.asarray(alphas, dtype=np.float64).reshape(5)
    a = np.exp(al - al.max())
    a = a / a.sum()
    M = np.einsum("i,iab->ab", a, make_masks())  # [256, 256], symmetric
    mc = np.ascontiguousarray(M.reshape(2, 128, 256)).astype(BF16_NP)

    ident = np.eye(128, dtype=np.float32).astype(BF16_NP)

    # per-token-half pre-chunked x^T sources: [KCH, TOKS, 128]
    xt_halves = []
    for th in range(T_SHARD):
        xh = x_bf[th * TOKS : (th + 1) * TOKS]  # [TOKS, 1024]
        xt = np.ascontiguousarray(
            xh.reshape(TOKS, KCH, 128).transpose(1, 0, 2)
        )  # [KCH, TOKS, 128]
        xt_halves.append(xt)

    in_maps = []
    for c in range(NCORES):
        th, oq = c // O_SHARD, c % O_SHARD
        in_maps.append(
            {
                "xt": xt_halves[th],
                "w": np.ascontiguousarray(weight_bf[oq * OSH : (oq + 1) * OSH]),
                "mc": mc,
                "bias": np.ascontiguousarray(
                    bias[oq * OSH : (oq + 1) * OSH]
                ).reshape(1, OSH),
                "ident": ident,
            }
        )
    return in_maps


def run(x, weight, alphas, bias, trace=False, **rkw):
    nc = _get_nc()
    in_maps = make_in_maps(x, weight, alphas, bias)
    res = run_bass_kernel_spmd(nc, in_maps, list(range(NCORES)), trace=trace, **rkw)
    y = np.empty((TOK, OUT_F), dtype=np.float32)
    for c in range(NCORES):
        th, oq = c // O_SHARD, c % O_SHARD
        y[th * TOKS : (th + 1) * TOKS, oq * OSH : (oq + 1) * OSH] = res.results[c]["y"]
    return y.reshape(16, 512, OUT_F), res


def kernel(x, weight, alphas, bias):
    y, _ = run(x, weight, alphas, bias)
    return y.astype(np.float32)


if __name__ == "__main__":
    rng = np.random.default_rng(0)
    x = rng.standard_normal((16, 512, IN_F), dtype=np.float32)
    w = (rng.standard_normal((OUT_F, IN_F)) * 0.02).astype(np.float32)
    a = rng.standard_normal(5).astype(np.float32)
    b = (rng.standard_normal(OUT_F) * 0.02).astype(np.float32)
    y = kernel(x=x, weight=w, alphas=a, bias=b)
    print("y", y.shape, y.dtype, float(np.abs(y).max()))
